# revision 32
# baseline (speedup 1.0000x reference)
"""GroupedQueryAttention kernel for 8 Trainium2 NeuronCores.

Sharding: core c = (batch b = c//2, seq-half sh = c%2). Each core computes the
full attention output for 1024 query rows of one batch: all 8 q heads
(2 kv heads), plus the q/k/v projections and the o-projection for those rows.

On-device layout: scoresT [keys, queries] so softmax-exp'd probabilities feed
attn@v matmuls directly as the moving operand.

The kernel is co-bound: the PE streams ~131us of matmuls per core while the
exp of H*SQ*S = 16.8M scores costs ~133us on the Scalar engine alone. To get
under the PE wall, exp is SPLIT between the Scalar engine (activation
table, exact) and the Vector engine, which computes a Schraudolph-style
approximate exp in ONE tensor_scalar op: e_bits = int16(A*score + B) where
A = 128*SCALE/ln2 and B = 16256 - 5.5; the int16 bit pattern IS the bf16
exp value (DVE output convert rounds-to-nearest; verified on HW). The ~2%
per-element error averages out below 2e-2 in the final output.

Other structure:
- vpt layout [128, 130]: cols 0:64 = v_kv0, 64 = ones, 65:129 = v_kv1,
  129 = ones. The ones columns make PSUM row 64 the softmax denominator
  for free.
- bv never touches the device: attn_out@(v+bv) = attn_out@v + bv, so
  bv_cat @ Wo is folded into the o-proj bias on the host; the v epilogue
  is one strided Scalar-engine copy.
- kt/q bias epilogues run on the Scalar engine (activation Identity with a
  per-partition bias AP) in the slots where the DVE produces exp.
- The normalize chain is staged: the [65, 512] accumulator halves are
  copied to SBUF immediately (pacc is effectively single-buffered -- the
  next job's attnv stalls otherwise), then den-shift / reciprocal /
  broadcast / muls spread over the following job's slots so no in-order
  engine stream blocks on the chain's cross-engine latency.
- For the last two jobs the attnv lag is shortened, and the final job
  skips the ot-assembly DMA: its o-proj pr3 runs as two 64-contraction
  matmuls against a partition-shifted copy of wo pair 3.
- Input DMAs are ordered critical-first across the sync/scalar/gpsimd
  queues (x chunk 0 and the pair-0 slice of Wq land first); a dummy exp
  preloads the activation table during the DMA wait.
"""

import numpy as np

B, S, D = 4, 2048, 512
H, KV, DH = 8, 2, 64
SQ = S // 2  # queries per core
NCORES = 8
PAIRS = 4  # head pairs (p, p+4); p -> kv0, p+4 -> kv1
SCALE = 1.0 / 8.0  # 1/sqrt(DH)
PERM = [0, 4, 1, 5, 2, 6, 3, 7]  # q head order: pair-major
NKB = S // 128  # 16 key blocks
NSC = S // 512  # 4 column chunks of x

# Schraudolph bf16 exp constants (DVE float->int16 convert rounds to nearest)
EXP_A = 128.0 / float(np.log(2.0)) * SCALE
EXP_B = 16256.0 - 5.5
# key blocks whose exp runs on the Vector engine (per job)
DVE_SET = frozenset((4, 6, 8, 10, 12, 14))

_built = {}
DEBUG_DUMP = False


def _build_nc():
    import concourse.mybir as mybir
    import concourse.tile as tile
    from concourse import bacc
    from concourse.alu_op_type import AluOpType

    fp32 = mybir.dt.float32
    bf16 = mybir.dt.bfloat16
    i16 = mybir.dt.int16
    Exp = mybir.ActivationFunctionType.Exp
    Ident = mybir.ActivationFunctionType.Identity

    nc = bacc.Bacc("TRN2", target_bir_lowering=False, debug=False,
                   num_devices=NCORES)

    # all matrices arrive pre-arranged on the host into the exact SBUF
    # layout [partition, chunk, col] so every input DMA is fully contiguous
    xp = nc.dram_tensor("xp", [128, NSC * 4 * 512], bf16,
                        kind="ExternalInput").ap()
    wq = nc.dram_tensor("wq", [128, 4 * D], bf16, kind="ExternalInput").ap()
    wk = nc.dram_tensor("wk", [128, 4 * 128], bf16, kind="ExternalInput").ap()
    wv = nc.dram_tensor("wv", [128, 4 * 128], bf16, kind="ExternalInput").ap()
    wo = nc.dram_tensor("wo", [128, 4 * D], bf16, kind="ExternalInput").ap()
    bqk = nc.dram_tensor("bqk", [128, PAIRS + 1], fp32,
                         kind="ExternalInput").ap()
    bobc = nc.dram_tensor("bobc", [128, D], fp32, kind="ExternalInput").ap()
    y = nc.dram_tensor("y", [SQ, D], fp32, kind="ExternalOutput").ap()

    with tile.TileContext(nc) as tc:
        with (
            tc.tile_pool(name="consts", bufs=1) as consts,
            tc.tile_pool(name="epool", bufs=5) as epool,
            tc.tile_pool(name="opool", bufs=9) as opool,
            tc.tile_pool(name="obpool", bufs=3) as obpool,
            tc.tile_pool(name="cpool", bufs=2) as cpool,
            tc.tile_pool(name="npool", bufs=3) as npool,
            tc.tile_pool(name="bcpool", bufs=4) as bcpool,
            tc.tile_pool(name="ypool", bufs=3) as ypool,
            tc.tile_pool(name="pssc", bufs=2, space="PSUM") as pssc,
            tc.tile_pool(name="pacc", bufs=2, space="PSUM") as pacc,
            tc.tile_pool(name="pproj", bufs=2, space="PSUM") as pproj,
        ):
            xt_ch = [consts.tile([128, 4, 512], bf16, name=f"xch{sc}",
                                 tag=f"xt{sc}") for sc in range(NSC)]
            wk_sb = consts.tile([128, 4, 128], bf16, tag="wk")
            wv_sb = consts.tile([128, 4, 128], bf16, tag="wv")
            wq0_sb = consts.tile([128, 4, 128], bf16, tag="wq0")
            wqr_sb = consts.tile([128, 4, 384], bf16, tag="wqr")
            wo_sb = consts.tile([128, 4, D], bf16, tag="wo")
            bqk_sb = consts.tile([128, PAIRS + 1], fp32, tag="bqk")
            bo_sb = consts.tile([128, D], fp32, tag="bo")
            dummy = consts.tile([1, 2], fp32, tag="dummy")

            # ---- prologue DMAs: critical-first across three queues.
            # sync: wk, x0 chunks 0-1, then (in-loop) half the transposes.
            # scalar: x0 chunks 2-3, wq pair-0 cols, wv, x1, x3, wq rest,
            #   x2 second half... (see below); gpsimd: biases, x2.
            xp3 = xp.rearrange("p (sc c j) -> p sc c j", sc=4, c=4)
            wq3 = wq.rearrange("p (c j) -> p c j", c=4)
            nc.sync.dma_start(wk_sb[:], wk.rearrange("p (c j) -> p c j", c=4))
            nc.sync.dma_start(xt_ch[0][:, 0:2, :], xp3[:, 0, 0:2, :])
            nc.scalar.dma_start(xt_ch[0][:, 2:4, :], xp3[:, 0, 2:4, :])
            nc.scalar.dma_start(wq0_sb[:], wq3[:, :, 0:128])
            nc.scalar.dma_start(wv_sb[:], wv.rearrange("p (c j) -> p c j", c=4))
            nc.gpsimd.dma_start(bqk_sb[:], bqk)
            nc.gpsimd.dma_start(xt_ch[2][:], xp3[:, 2])
            nc.sync.dma_start(xt_ch[1][:], xp3[:, 1])
            nc.sync.dma_start(xt_ch[3][:], xp3[:, 3])
            nc.sync.dma_start(wqr_sb[:], wq3[:, :, 128:512])
            nc.sync.dma_start(wo_sb[:], wo.rearrange("p (c j) -> p c j", c=4))
            nc.sync.dma_start(bo_sb[:], bobc)
            # pair-3 wo rows 64:128 shifted to partitions 0:64: lets the
            # final o-proj consume job 7's unassembled attn-out half with
            # both matmul operands at base partition 0
            wo3b = consts.tile([64, 512], bf16, tag="wo3b")
            nc.sync.dma_start(wo3b[:], wo_sb[64:128, 3, :])
            # preload the exp activation table while DMAs are in flight
            nc.vector.memset(dummy[:], 0.0)
            nc.scalar.activation(dummy[0:1, 0:1], dummy[0:1, 1:2], Exp)

            # per-chunk kT / vT tiles ([128 dims, 512 keys])
            ktt = [consts.tile([128, 512], bf16, name=f"ktt{sc}",
                               tag=f"kt{sc}") for sc in range(NSC)]
            # V stationary per key block: cols 0:64 = v0, 64 = ones,
            # 65:129 = v1, 129 = ones; the ones columns make PSUM row 64
            # the softmax denominator for free.
            vpt = [consts.tile([128, 130], bf16, name=f"vpt{kb}",
                               tag=f"vp{kb}") for kb in range(NKB)]
            qtt = [[consts.tile([128, 512], bf16, name=f"qtt{pr}_{qc}",
                                tag=f"qt{pr}_{qc}") for qc in range(2)]
                   for pr in range(PAIRS)]
            for kb in range(NKB):
                nc.gpsimd.memset(vpt[kb][:, 64:65], 1.0)
                nc.gpsimd.memset(vpt[kb][:, 129:130], 1.0)

            # Projection emitters, split into <=2-matmul pieces drained into
            # the PE idle gaps of the attention loop ("deferred work").
            def kt_mm(sc, cs, box):
                if "ps" not in box:
                    box["ps"] = pproj.tile([128, 512], fp32, name=f"pk{sc}",
                                           tag="pproj")
                ps = box["ps"]
                for c in (cs, cs + 1):
                    nc.tensor.matmul(ps[:], wk_sb[:, c, :],
                                     xt_ch[sc][:, c, :],
                                     start=(c == 0), stop=(c == 3))
                if cs == 2:
                    nc.scalar.activation(ktt[sc][:], ps[:], Ident,
                                         bias=bqk_sb[:, PAIRS:PAIRS + 1])

            def v_mm(kb, cs, box):
                if "ps" not in box:
                    box["ps"] = pproj.tile([128, 512], fp32, name=f"pv{kb}",
                                           tag="pproj")
                ps = box["ps"]
                xch = xt_ch[kb // 4]
                off = (kb % 4) * 128
                for c in (cs, cs + 1):
                    nc.tensor.matmul(ps[:, 0:128],
                                     xch[:, c, off:off + 128],
                                     wv_sb[:, c, :],
                                     start=(c == 0), stop=(c == 3))
                if cs == 2:
                    # one strided copy splits [keys, 128 vd] around the
                    # ones columns of the vpt tile (bv is folded into the
                    # o-proj bias on the host, so no adds here)
                    dst = vpt[kb][:].rearrange(
                        "p (c j) -> p c j", c=2)[:, :, 0:64]
                    nc.scalar.copy(dst, ps[:, 0:128].rearrange(
                        "p (c j) -> p c j", c=2))

            def qt_mm(pr, qc, cs, box):
                if "ps" not in box:
                    box["ps"] = pproj.tile([128, 512], fp32, name=f"pq{pr}{qc}",
                                           tag="pproj")
                ps = box["ps"]
                for c in (cs, cs + 1):
                    wsl = (wq0_sb[:, c, :] if pr == 0 else
                           wqr_sb[:, c, (pr - 1) * 128:pr * 128])
                    nc.tensor.matmul(ps[:], wsl,
                                     xt_ch[qc][:, c, :],
                                     start=(c == 0), stop=(c == 3))
                if cs == 2:
                    nc.scalar.activation(qtt[pr][qc][:], ps[:], Ident,
                                         bias=bqk_sb[:, pr:pr + 1])

            ot_tiles = {}  # (qc, pr) -> assembled [128, 512] bf16 attn out

            def oproj_mm(qc, m, prs, box):
                if "ps" not in box:
                    box["ps"] = pproj.tile([128, 512], fp32, name=f"po{qc}{m}",
                                           tag="pproj")
                ps = box["ps"]
                for pr in (prs, prs + 1):
                    nc.tensor.matmul(ps[:],
                                     ot_tiles[(qc, pr)][:, m * 128:(m + 1) * 128],
                                     wo_sb[:, pr, :],
                                     start=(pr == 0), stop=(pr == 3))

            def oproj_fin(qc, m, box):
                yt = ypool.tile([128, 512], fp32, name=f"yt{qc}{m}", tag="y")
                nc.vector.tensor_add(yt[:], box["ps"][:], bo_sb[:])
                blk = qc * 4 + m
                nc.sync.dma_start(y[blk * 128:(blk + 1) * 128, :], yt[:])

            def chain(fn, *idx):
                box = {}
                fn(*idx, 0, box)
                fn(*idx, 2, box)
                return box

            # ---- serial prologue: the minimum before exp can start ----
            chain(kt_mm, 0)       # kT chunk 0   (wk + x0)
            chain(qt_mm, 0, 0)    # qT pair0 half0  (wq pair-0 cols + x0)
            chain(v_mm, 0)        # V block 0 (attnv is 2 deep; v1/v2
                                  # drain in slots 0/1)

            # deferred 2-matmul pieces keyed by GLOBAL slot s = 16*j + kb
            deferred = {}
            boxes = {}

            def defer(s, key, fn, *idx):
                box = boxes.setdefault(key, {})
                deferred.setdefault(s, []).append(
                    (lambda b: (lambda: fn(*idx, b)))(box))

            # remaining V blocks (vp(k) needed by attnv(k) at slot k+2)
            # and kT chunks 1-3 (ktt[c] needed by scores at slot 4c)
            defer(0, "v1", v_mm, 1, 0)
            defer(0, "v1", v_mm, 1, 2)
            defer(0, "k1", kt_mm, 1, 0)
            defer(0, "k1", kt_mm, 1, 2)
            defer(1, "v2", v_mm, 2, 0)
            defer(1, "v2", v_mm, 2, 2)
            defer(1, "v3", v_mm, 3, 0)
            defer(1, "v3", v_mm, 3, 2)
            vslot = {4: 2, 5: 3, 6: 4, 7: 6, 8: 7, 9: 8, 10: 10, 11: 11,
                     12: 12, 13: 13, 14: 14, 15: 15}
            for k in range(4, NKB):
                defer(vslot[k], f"v{k}", v_mm, k, 0)
                defer(vslot[k], f"v{k}", v_mm, k, 2)
            defer(5, "k2", kt_mm, 2, 0)
            defer(5, "k2", kt_mm, 2, 2)
            defer(9, "k3", kt_mm, 3, 0)
            defer(9, "k3", kt_mm, 3, 2)
            defer(15, "q01", qt_mm, 0, 1, 0)
            defer(15, "q01", qt_mm, 0, 1, 2)
            # qT for the next pair drains across the two jobs before it
            for pr in range(1, PAIRS):
                s0 = (2 * pr - 1) * 16
                defer(s0 + 4, f"q{pr}0", qt_mm, pr, 0, 0)
                defer(s0 + 5, f"q{pr}0", qt_mm, pr, 0, 2)
                defer(s0 + 8, f"q{pr}1", qt_mm, pr, 1, 0)
                defer(s0 + 9, f"q{pr}1", qt_mm, pr, 1, 2)
            # o-proj for qc0 hides in job 7 (ot(0,3) assembles ~slot 116)
            for m in range(4):
                a, b, f = ((117, 118, 119), (120, 121, 122),
                           (123, 124, 125), (125, 126, 127))[m]
                defer(a, f"o{m}", oproj_mm, 0, m, 0)
                defer(b, f"o{m}", oproj_mm, 0, m, 2)
                defer(f, f"o{m}", oproj_fin, 0, m)

            # ---- flat attention pipeline: 8 jobs x 16 key blocks ----
            jobs = [(qc, pr) for pr in range(PAIRS) for qc in range(2)]
            job_state = {}  # j -> (pA, pB, e_tiles)

            def attnv(j, kb):
                pA, pB, e_tiles = job_state[j]
                e = e_tiles[kb]
                nc.tensor.matmul(pA[0:65, :], vpt[kb][:, 0:65],
                                 e[:, 0:512],
                                 start=(kb == 0), stop=(kb == NKB - 1))
                nc.tensor.matmul(pB[0:65, :], vpt[kb][:, 65:130],
                                 e[:, 512:1024],
                                 start=(kb == 0), stop=(kb == NKB - 1))

            # Normalize pipeline.  pacc is effectively single-buffered per
            # tag, so stage 0 (emitted inline right after the job's last
            # attnv) copies the live [65, 512] accumulator halves to SBUF
            # immediately -- the next job's attnv would otherwise stall on
            # the PSUM buffers.  The rest of the chain (den row to
            # partition 0 by DMA, reciprocal there -- the custom DVE op
            # misbehaves off partition 0 on HW -- GPSIMD partition
            # broadcast, normalize muls) is spread over the following
            # job's slots so no in-order engine stream blocks on its
            # cross-engine latency.
            norm_state = {}

            def norm_s0(j):
                pA, pB, _ = job_state[j]
                cpA = cpool.tile([65, 512], fp32, tag="cpA")
                cpB = cpool.tile([65, 512], fp32, tag="cpB")
                nc.vector.tensor_copy(cpA[:], pA[0:65, :])
                nc.vector.tensor_copy(cpB[:], pB[0:65, :])
                norm_state[j] = [cpA, cpB]

            def norm_s1(j):
                cpA, cpB = norm_state[j]
                d0 = npool.tile([1, 1024], fp32, tag="den0")
                nc.sync.dma_start(d0[0:1, 0:512], cpA[64:65, :])
                nc.sync.dma_start(d0[0:1, 512:1024], cpB[64:65, :])
                norm_state[j] = [cpA, cpB, d0]

            def norm_s2(j):
                cpA, cpB, d0 = norm_state[j]
                r0 = npool.tile([1, 1024], fp32, tag="rden0")
                nc.vector.reciprocal_approx_fast(out=r0[:], in_=d0[:])
                rbcA = bcpool.tile([64, 512], fp32, tag="rbcA")
                rbcB = bcpool.tile([64, 512], fp32, tag="rbcB")
                nc.gpsimd.partition_broadcast(rbcA[:], r0[0:1, 0:512],
                                              channels=64)
                nc.gpsimd.partition_broadcast(rbcB[:], r0[0:1, 512:1024],
                                              channels=64)
                norm_state[j] = [cpA, cpB, rbcA, rbcB]

            def norm_s3(j, last):
                qc, pr = jobs[j]
                cpA, cpB, rbcA, rbcB = norm_state[j]
                ot = opool.tile([128, 512], bf16, tag="ot")
                nc.vector.tensor_mul(ot[0:64, :], cpA[0:64, :], rbcA[:])
                obt = obpool.tile([64, 512], bf16, tag="obt")
                nc.vector.tensor_mul(obt[:], cpB[0:64, :], rbcB[:])
                if not last:
                    # assemble both halves into one [128, 512] tile via an
                    # SBUF->SBUF DMA partition shift so o-proj contracts
                    # over all 128 dims at once
                    nc.sync.dma_start(ot[64:128, :], obt[:])
                    ot_tiles[(qc, pr)] = ot
                else:
                    ot_tiles[(qc, pr)] = (ot, obt)  # unassembled halves

            # normalize stage schedule: jobs 0-5 spread over the next job's
            # slots (chain latency hidden, no engine blocks); job 6 tighter
            # (o-proj qc0 drains need ot(0,3) by slot ~117); job 7 at tail.
            norm_sched = {}
            for j in range(6):
                norm_sched[16 * j + 18] = [
                    (lambda jj: (lambda: norm_s1(jj)))(j)]
                norm_sched[16 * j + 19] = [
                    (lambda jj: (lambda: norm_s2(jj)))(j)]
                norm_sched[16 * j + 21] = [
                    (lambda jj: (lambda: norm_s3(jj, False)))(j)]
            norm_sched[113] = [lambda: norm_s1(6)]
            norm_sched[114] = [lambda: norm_s2(6)]
            norm_sched[115] = [lambda: norm_s3(6, False)]

            for s in range(128):
                j, kb = divmod(s, 16)
                qc, pr = jobs[j]
                if kb == 0:
                    pA = pacc.tile([128, 512], fp32, tag="pacc")
                    pB = pacc.tile([128, 512], fp32, tag="pacc")
                    job_state[j] = (pA, pB, [None] * NKB)

                sc_ps = pssc.tile([128, 1024], fp32, tag="scores")
                nc.tensor.matmul(
                    sc_ps[:, 0:512],
                    ktt[kb // 4][0:64, (kb % 4) * 128:(kb % 4 + 1) * 128],
                    qtt[pr][qc][0:64, :])
                nc.tensor.matmul(
                    sc_ps[:, 512:1024],
                    ktt[kb // 4][64:128, (kb % 4) * 128:(kb % 4 + 1) * 128],
                    qtt[pr][qc][64:128, :])
                e = epool.tile([128, 1024], bf16, tag="E")
                job_state[j][2][kb] = e
                if kb in DVE_SET:
                    nc.vector.tensor_scalar(e[:].bitcast(i16), sc_ps[:],
                                            EXP_A, EXP_B,
                                            AluOpType.mult, AluOpType.add)
                else:
                    nc.scalar.activation(e[:], sc_ps[:], Exp, scale=SCALE)
                if DEBUG_DUMP and s == 0:
                    dbg_e_tile = consts.tile([128, 1024], fp32, tag="dbge")
                    nc.vector.tensor_copy(dbg_e_tile[:], e[:])

                # consume probs with a 2-slot lag so the PE never waits on
                # exp; jobs 6/7 shorten the lag at the job end so their
                # normalize chains (which gate o-proj) start earlier.
                special = {(6, 13), (6, 14), (6, 15),
                           (7, 13), (7, 14), (7, 15)}
                if s >= 2:
                    j2, kb2 = divmod(s - 2, 16)
                    if (j2, kb2) not in special:
                        if kb2 == 0 and j2 >= 1:
                            pass  # deferred one slot (pacc copies in flight)
                        elif kb2 == 1 and j2 >= 1:
                            attnv(j2, 0)
                            attnv(j2, 1)
                        else:
                            attnv(j2, kb2)
                        if kb2 == 15:
                            norm_s0(j2)  # free the PSUM accumulators
                if kb == 15 and j >= 6:
                    attnv(j, 13)  # 2-lag
                    attnv(j, 14)  # 1-lag
                if s == 112:
                    attnv(6, 15)  # 0-lag
                    norm_s0(6)

                for fn in deferred.get(s, ()):
                    fn()
                for fn in norm_sched.get(s, ()):
                    fn()

            # tail: finish job 7, then o-proj for qc1
            attnv(7, 15)
            norm_s0(7)
            norm_s1(7)
            norm_s2(7)
            norm_s3(7, last=True)
            otA, otB = ot_tiles[(1, 3)]

            tb = {}
            for m in range(4):
                tb[m] = box = {}
                if m >= 2:
                    box["ps"] = pacc.tile([128, 512], fp32,
                                          name=f"ypt{m}", tag="pacc")
                oproj_mm(1, m, 0, box)  # pr 0,1
            for m in range(4):
                ps = tb[m]["ps"]
                nc.tensor.matmul(ps[:],
                                 ot_tiles[(1, 2)][:, m * 128:(m + 1) * 128],
                                 wo_sb[:, 2, :], start=False, stop=False)
                # pr3 from the unassembled halves as two 64-row matmuls
                # (wo3b is the partition-shifted copy of wo pair-3 rows
                # 64:128) -- skips the ot-assembly DMA on the critical tail
                nc.tensor.matmul(ps[:],
                                 otA[0:64, m * 128:(m + 1) * 128],
                                 wo_sb[0:64, 3, :], start=False, stop=False)
                nc.tensor.matmul(ps[:],
                                 otB[0:64, m * 128:(m + 1) * 128],
                                 wo3b[:], start=False, stop=True)
                oproj_fin(1, m, tb[m])

            if DEBUG_DUMP:
                dbg_vpt = nc.dram_tensor("dbg_vpt", [128, 130], fp32,
                                         kind="ExternalOutput").ap()
                dbg_kt = nc.dram_tensor("dbg_kt", [128, 512], fp32,
                                        kind="ExternalOutput").ap()
                dbg_qt = nc.dram_tensor("dbg_qt", [128, 512], fp32,
                                        kind="ExternalOutput").ap()
                dbg_ot = nc.dram_tensor("dbg_ot", [128, 512], fp32,
                                        kind="ExternalOutput").ap()
                dbg_e = nc.dram_tensor("dbg_e", [128, 1024], fp32,
                                       kind="ExternalOutput").ap()
                tmp = consts.tile([128, 1024], fp32, tag="dbgtmp")
                nc.vector.tensor_copy(tmp[:, 0:130], vpt[0][:])
                nc.sync.dma_start(dbg_vpt, tmp[:, 0:130])
                nc.vector.tensor_copy(tmp[:, 0:512], ktt[0][:])
                nc.sync.dma_start(dbg_kt, tmp[:, 0:512])
                nc.vector.tensor_copy(tmp[:, 0:512], qtt[0][0][:])
                nc.sync.dma_start(dbg_qt, tmp[:, 0:512])
                nc.vector.tensor_copy(tmp[:, 0:512], ot_tiles[(0, 0)][:])
                nc.sync.dma_start(dbg_ot, tmp[:, 0:512])
                nc.vector.tensor_copy(tmp[:], dbg_e_tile[:])
                nc.sync.dma_start(dbg_e, tmp[:])

    nc.finalize()
    return nc


def _get_nc():
    if "nc" not in _built:
        _built["nc"] = _build_nc()
    return _built["nc"]


def _in_maps(x, Wq, bq, Wk, bk, Wv, bv, Wo, bo):
    import ml_dtypes

    b16 = ml_dtypes.bfloat16
    x = np.ascontiguousarray(np.asarray(x, np.float32))
    Wq = np.asarray(Wq, np.float32)
    bq = np.asarray(bq, np.float32)
    Wk = np.asarray(Wk, np.float32)
    bk = np.asarray(bk, np.float32)
    Wv = np.asarray(Wv, np.float32)
    bv = np.asarray(bv, np.float32)
    Wo = np.asarray(Wo, np.float32)
    bo = np.asarray(bo, np.float32)

    def chunked(a):  # [D, n] row-major -> [128, 4*n] with row d = (c, p)
        n = a.shape[1]
        return np.ascontiguousarray(
            a.reshape(4, 128, n).transpose(1, 0, 2).reshape(128, 4 * n))

    wq_p = chunked(
        Wq.reshape(D, H, DH)[:, PERM, :].reshape(D, D)).astype(b16)
    wo_p = chunked(
        Wo.reshape(H, DH, D)[PERM].reshape(D, D)).astype(b16)
    wk_p = chunked(Wk).astype(b16)
    wv_p = chunked(Wv).astype(b16)
    bq_p = np.ascontiguousarray(
        bq.reshape(H, DH)[PERM].reshape(PAIRS, 128).T)
    bqk_p = np.ascontiguousarray(
        np.concatenate([bq_p, bk.reshape(128, 1)], axis=1))
    # fold the v bias through the o-projection: attn@(v+bv) = attn@v + bv
    bv_cat = np.concatenate([bv[(h // 4) * DH:(h // 4 + 1) * DH]
                             for h in range(H)])
    bo_eff = bo + bv_cat @ Wo
    bo_bc = np.ascontiguousarray(np.tile(bo_eff[None, :], (128, 1)))

    in_maps = []
    for c in range(NCORES):
        b, sh = divmod(c, 2)
        xroll = np.roll(x[b], -sh * SQ, axis=0)
        # xT [D, S] -> [128, sc, c, 512] chunk-major contiguous
        xprep = np.ascontiguousarray(
            xroll.T.reshape(4, 128, 4, 512).transpose(1, 2, 0, 3)
            .reshape(128, NSC * 4 * 512)).astype(b16)
        in_maps.append({
            "xp": xprep,
            "wq": wq_p, "wk": wk_p, "wv": wv_p, "wo": wo_p,
            "bqk": bqk_p, "bobc": bo_bc,
        })
    return in_maps


def kernel(x, Wq, bq, Wk, bk, Wv, bv, Wo, bo):
    from concourse.bass_utils import run_bass_kernel_spmd

    in_maps = _in_maps(x, Wq, bq, Wk, bk, Wv, bv, Wo, bo)
    nc = _get_nc()
    res = run_bass_kernel_spmd(nc, in_maps, list(range(NCORES)))
    out = np.empty((B, S, D), np.float32)
    for c in range(NCORES):
        b, sh = divmod(c, 2)
        out[b, sh * SQ:(sh + 1) * SQ, :] = res.results[c]["y"]
    return out


# revision 35
# speedup vs baseline: 1.0097x; 1.0097x over previous
"""GroupedQueryAttention kernel for 8 Trainium2 NeuronCores.

Sharding: core c = (batch b = c//2, seq-half sh = c%2). Each core computes the
full attention output for 1024 query rows of one batch: all 8 q heads
(2 kv heads), plus the q/k/v projections and the o-projection for those rows.

On-device layout: scoresT [keys, queries] so softmax-exp'd probabilities feed
attn@v matmuls directly as the moving operand.

The kernel is co-bound: the PE streams ~131us of matmuls per core while the
exp of H*SQ*S = 16.8M scores costs ~133us on the Scalar engine alone. To get
under the PE wall, exp is SPLIT between the Scalar engine (activation
table, exact) and the Vector engine, which computes a Schraudolph-style
approximate exp in ONE tensor_scalar op: e_bits = int16(A*score + B) where
A = 128*SCALE/ln2 and B = 16256 - 5.5; the int16 bit pattern IS the bf16
exp value (DVE output convert rounds-to-nearest; verified on HW). The ~2%
per-element error averages out below 2e-2 in the final output.

Other structure:
- vpt layout [128, 130]: cols 0:64 = v_kv0, 64 = ones, 65:129 = v_kv1,
  129 = ones. The ones columns make PSUM row 64 the softmax denominator
  for free.
- bv never touches the device: attn_out@(v+bv) = attn_out@v + bv, so
  bv_cat @ Wo is folded into the o-proj bias on the host; the v epilogue
  is one strided Scalar-engine copy.
- kt/q bias epilogues run on the Scalar engine (activation Identity with a
  per-partition bias AP) in the slots where the DVE produces exp.
- The normalize chain is staged: the [65, 512] accumulator halves are
  copied to SBUF immediately (pacc is effectively single-buffered -- the
  next job's attnv stalls otherwise), then den-shift / reciprocal /
  broadcast / muls spread over the following job's slots so no in-order
  engine stream blocks on the chain's cross-engine latency.
- For the last two jobs the attnv lag is shortened, and the final job
  skips the ot-assembly DMA: its o-proj pr3 runs as two 64-contraction
  matmuls against a partition-shifted copy of wo pair 3.
- Input DMAs are ordered critical-first across the sync/scalar/gpsimd
  queues (x chunk 0 and the pair-0 slice of Wq land first); a dummy exp
  preloads the activation table during the DMA wait.
"""

import numpy as np

B, S, D = 4, 2048, 512
H, KV, DH = 8, 2, 64
SQ = S // 2  # queries per core
NCORES = 8
PAIRS = 4  # head pairs (p, p+4); p -> kv0, p+4 -> kv1
SCALE = 1.0 / 8.0  # 1/sqrt(DH)
PERM = [0, 4, 1, 5, 2, 6, 3, 7]  # q head order: pair-major
NKB = S // 128  # 16 key blocks
NSC = S // 512  # 4 column chunks of x

# Schraudolph bf16 exp constants (DVE float->int16 convert rounds to nearest)
EXP_A = 128.0 / float(np.log(2.0)) * SCALE
EXP_B = 16256.0 - 5.5
# key blocks whose exp runs on the Vector engine (per job)
DVE_SET = frozenset((4, 6, 8, 10, 12, 14))

_built = {}
DEBUG_DUMP = False


def _build_nc():
    import concourse.mybir as mybir
    import concourse.tile as tile
    from concourse import bacc
    from concourse.alu_op_type import AluOpType

    fp32 = mybir.dt.float32
    bf16 = mybir.dt.bfloat16
    i16 = mybir.dt.int16
    Exp = mybir.ActivationFunctionType.Exp
    Ident = mybir.ActivationFunctionType.Identity

    nc = bacc.Bacc("TRN2", target_bir_lowering=False, debug=False,
                   num_devices=NCORES)

    # all matrices arrive pre-arranged on the host into the exact SBUF
    # layout [partition, chunk, col] so every input DMA is fully contiguous
    xp = nc.dram_tensor("xp", [128, NSC * 4 * 512], bf16,
                        kind="ExternalInput").ap()
    wq = nc.dram_tensor("wq", [128, 4 * D], bf16, kind="ExternalInput").ap()
    wk = nc.dram_tensor("wk", [128, 4 * 128], bf16, kind="ExternalInput").ap()
    wv = nc.dram_tensor("wv", [128, 4 * 128], bf16, kind="ExternalInput").ap()
    wo = nc.dram_tensor("wo", [128, 4 * D], bf16, kind="ExternalInput").ap()
    bqk = nc.dram_tensor("bqk", [128, PAIRS + 1], fp32,
                         kind="ExternalInput").ap()
    bobc = nc.dram_tensor("bobc", [128, D], fp32, kind="ExternalInput").ap()
    y = nc.dram_tensor("y", [SQ, D], fp32, kind="ExternalOutput").ap()

    with tile.TileContext(nc) as tc:
        with (
            tc.tile_pool(name="consts", bufs=1) as consts,
            tc.tile_pool(name="epool", bufs=5) as epool,
            tc.tile_pool(name="opool", bufs=9) as opool,
            tc.tile_pool(name="obpool", bufs=3) as obpool,
            tc.tile_pool(name="cpool", bufs=2) as cpool,
            tc.tile_pool(name="npool", bufs=3) as npool,
            tc.tile_pool(name="bcpool", bufs=4) as bcpool,
            tc.tile_pool(name="ypool", bufs=3) as ypool,
            tc.tile_pool(name="pssc", bufs=2, space="PSUM") as pssc,
            tc.tile_pool(name="pacc", bufs=2, space="PSUM") as pacc,
            tc.tile_pool(name="pproj", bufs=2, space="PSUM") as pproj,
        ):
            xt_ch = [consts.tile([128, 4, 512], bf16, name=f"xch{sc}",
                                 tag=f"xt{sc}") for sc in range(NSC)]
            wk_sb = consts.tile([128, 4, 128], bf16, tag="wk")
            wv_sb = consts.tile([128, 4, 128], bf16, tag="wv")
            wq0_sb = consts.tile([128, 4, 128], bf16, tag="wq0")
            wqr_sb = consts.tile([128, 4, 384], bf16, tag="wqr")
            wo_sb = consts.tile([128, 4, D], bf16, tag="wo")
            bqk_sb = consts.tile([128, PAIRS + 1], fp32, tag="bqk")
            bo_sb = consts.tile([128, D], fp32, tag="bo")
            dummy = consts.tile([1, 2], fp32, tag="dummy")

            # ---- prologue DMAs: critical-first across three queues.
            # sync: wk, x0 chunks 0-1, then (in-loop) half the transposes.
            # scalar: x0 chunks 2-3, wq pair-0 cols, wv, x1, x3, wq rest,
            #   x2 second half... (see below); gpsimd: biases, x2.
            xp3 = xp.rearrange("p (sc c j) -> p sc c j", sc=4, c=4)
            wq3 = wq.rearrange("p (c j) -> p c j", c=4)
            nc.sync.dma_start(wk_sb[:], wk.rearrange("p (c j) -> p c j", c=4))
            nc.sync.dma_start(xt_ch[0][:, 0:2, :], xp3[:, 0, 0:2, :])
            nc.scalar.dma_start(xt_ch[0][:, 2:4, :], xp3[:, 0, 2:4, :])
            nc.scalar.dma_start(wq0_sb[:], wq3[:, :, 0:128])
            nc.scalar.dma_start(wv_sb[:], wv.rearrange("p (c j) -> p c j", c=4))
            nc.gpsimd.dma_start(bqk_sb[:], bqk)
            nc.gpsimd.dma_start(xt_ch[2][:], xp3[:, 2])
            nc.scalar.dma_start(xt_ch[1][:], xp3[:, 1])
            nc.scalar.dma_start(xt_ch[3][:], xp3[:, 3])
            nc.scalar.dma_start(wqr_sb[:], wq3[:, :, 128:512])
            nc.gpsimd.dma_start(wo_sb[:], wo.rearrange("p (c j) -> p c j", c=4))
            nc.gpsimd.dma_start(bo_sb[:], bobc)
            # pair-3 wo rows 64:128 shifted to partitions 0:64: lets the
            # final o-proj consume job 7's unassembled attn-out half with
            # both matmul operands at base partition 0
            wo3b = consts.tile([64, 512], bf16, tag="wo3b")
            nc.sync.dma_start(wo3b[:], wo_sb[64:128, 3, :])
            # preload the exp activation table while DMAs are in flight
            nc.vector.memset(dummy[:], 0.0)
            nc.scalar.activation(dummy[0:1, 0:1], dummy[0:1, 1:2], Exp)
            # PE warm-up: spin dummy matmuls during the DMA wait so the
            # p-state throttle ramps before the first real chains, and a
            # dedicated PSUM scratch so they never alias real work
            wdum = consts.tile([128, 512], bf16, tag="wdum")
            nc.vector.memset(wdum[:], 0.0)

            def pe_warm(n):
                pw = pproj.tile([128, 512], fp32, tag="pproj")
                for _ in range(n):
                    nc.tensor.matmul(pw[0:1, :], wdum[:, 0:1], wdum[:],
                                     start=True, stop=True)

            # per-chunk kT / vT tiles ([128 dims, 512 keys])
            ktt = [consts.tile([128, 512], bf16, name=f"ktt{sc}",
                               tag=f"kt{sc}") for sc in range(NSC)]
            # V stationary per key block: cols 0:64 = v0, 64 = ones,
            # 65:129 = v1, 129 = ones; the ones columns make PSUM row 64
            # the softmax denominator for free.
            vpt = [consts.tile([128, 130], bf16, name=f"vpt{kb}",
                               tag=f"vp{kb}") for kb in range(NKB)]
            qtt = [[consts.tile([128, 512], bf16, name=f"qtt{pr}_{qc}",
                                tag=f"qt{pr}_{qc}") for qc in range(2)]
                   for pr in range(PAIRS)]
            for kb in range(NKB):
                nc.gpsimd.memset(vpt[kb][:, 64:65], 1.0)
                nc.gpsimd.memset(vpt[kb][:, 129:130], 1.0)

            # Projection emitters, split into <=2-matmul pieces drained into
            # the PE idle gaps of the attention loop ("deferred work").
            def kt_mm(sc, cs, box):
                if "ps" not in box:
                    box["ps"] = pproj.tile([128, 512], fp32, name=f"pk{sc}",
                                           tag="pproj")
                ps = box["ps"]
                for c in (cs, cs + 1):
                    nc.tensor.matmul(ps[:], wk_sb[:, c, :],
                                     xt_ch[sc][:, c, :],
                                     start=(c == 0), stop=(c == 3))
                if cs == 2:
                    nc.scalar.activation(ktt[sc][:], ps[:], Ident,
                                         bias=bqk_sb[:, PAIRS:PAIRS + 1])

            def v_mm(kb, cs, box):
                if "ps" not in box:
                    box["ps"] = pproj.tile([128, 512], fp32, name=f"pv{kb}",
                                           tag="pproj")
                ps = box["ps"]
                xch = xt_ch[kb // 4]
                off = (kb % 4) * 128
                for c in (cs, cs + 1):
                    nc.tensor.matmul(ps[:, 0:128],
                                     xch[:, c, off:off + 128],
                                     wv_sb[:, c, :],
                                     start=(c == 0), stop=(c == 3))
                if cs == 2:
                    # one strided copy splits [keys, 128 vd] around the
                    # ones columns of the vpt tile (bv is folded into the
                    # o-proj bias on the host, so no adds here)
                    dst = vpt[kb][:].rearrange(
                        "p (c j) -> p c j", c=2)[:, :, 0:64]
                    nc.scalar.copy(dst, ps[:, 0:128].rearrange(
                        "p (c j) -> p c j", c=2))

            def qt_mm(pr, qc, cs, box):
                if "ps" not in box:
                    box["ps"] = pproj.tile([128, 512], fp32, name=f"pq{pr}{qc}",
                                           tag="pproj")
                ps = box["ps"]
                for c in (cs, cs + 1):
                    wsl = (wq0_sb[:, c, :] if pr == 0 else
                           wqr_sb[:, c, (pr - 1) * 128:pr * 128])
                    nc.tensor.matmul(ps[:], wsl,
                                     xt_ch[qc][:, c, :],
                                     start=(c == 0), stop=(c == 3))
                if cs == 2:
                    nc.scalar.activation(qtt[pr][qc][:], ps[:], Ident,
                                         bias=bqk_sb[:, pr:pr + 1])

            ot_tiles = {}  # (qc, pr) -> assembled [128, 512] bf16 attn out

            def oproj_mm(qc, m, prs, box):
                if "ps" not in box:
                    box["ps"] = pproj.tile([128, 512], fp32, name=f"po{qc}{m}",
                                           tag="pproj")
                ps = box["ps"]
                for pr in (prs, prs + 1):
                    nc.tensor.matmul(ps[:],
                                     ot_tiles[(qc, pr)][:, m * 128:(m + 1) * 128],
                                     wo_sb[:, pr, :],
                                     start=(pr == 0), stop=(pr == 3))

            def oproj_fin(qc, m, box):
                yt = ypool.tile([128, 512], fp32, name=f"yt{qc}{m}", tag="y")
                nc.vector.tensor_add(yt[:], box["ps"][:], bo_sb[:])
                blk = qc * 4 + m
                nc.sync.dma_start(y[blk * 128:(blk + 1) * 128, :], yt[:])

            def chain(fn, *idx):
                box = {}
                fn(*idx, 0, box)
                fn(*idx, 2, box)
                return box

            # ---- serial prologue: the minimum before exp can start ----
            pe_warm(10)
            chain(kt_mm, 0)       # kT chunk 0   (wk + x0)
            chain(qt_mm, 0, 0)    # qT pair0 half0  (wq pair-0 cols + x0)
            chain(v_mm, 0)        # V block 0 (attnv is 2 deep; v1/v2
                                  # drain in slots 0/1)

            # deferred 2-matmul pieces keyed by GLOBAL slot s = 16*j + kb
            deferred = {}
            boxes = {}

            def defer(s, key, fn, *idx):
                box = boxes.setdefault(key, {})
                deferred.setdefault(s, []).append(
                    (lambda b: (lambda: fn(*idx, b)))(box))

            # remaining V blocks (vp(k) needed by attnv(k) at slot k+2)
            # and kT chunks 1-3 (ktt[c] needed by scores at slot 4c)
            defer(0, "v1", v_mm, 1, 0)
            defer(0, "v1", v_mm, 1, 2)
            defer(0, "k1", kt_mm, 1, 0)
            defer(0, "k1", kt_mm, 1, 2)
            defer(1, "v2", v_mm, 2, 0)
            defer(1, "v2", v_mm, 2, 2)
            defer(1, "v3", v_mm, 3, 0)
            defer(1, "v3", v_mm, 3, 2)
            vslot = {4: 2, 5: 3, 6: 4, 7: 6, 8: 7, 9: 8, 10: 10, 11: 11,
                     12: 12, 13: 13, 14: 14, 15: 15}
            for k in range(4, NKB):
                defer(vslot[k], f"v{k}", v_mm, k, 0)
                defer(vslot[k], f"v{k}", v_mm, k, 2)
            defer(5, "k2", kt_mm, 2, 0)
            defer(5, "k2", kt_mm, 2, 2)
            defer(9, "k3", kt_mm, 3, 0)
            defer(9, "k3", kt_mm, 3, 2)
            defer(15, "q01", qt_mm, 0, 1, 0)
            defer(15, "q01", qt_mm, 0, 1, 2)
            # qT for the next pair drains across the two jobs before it
            for pr in range(1, PAIRS):
                s0 = (2 * pr - 1) * 16
                defer(s0 + 4, f"q{pr}0", qt_mm, pr, 0, 0)
                defer(s0 + 5, f"q{pr}0", qt_mm, pr, 0, 2)
                defer(s0 + 8, f"q{pr}1", qt_mm, pr, 1, 0)
                defer(s0 + 9, f"q{pr}1", qt_mm, pr, 1, 2)
            # o-proj for qc0 hides in job 7 (ot(0,3) assembles ~slot 116)
            for m in range(4):
                a, b, f = ((117, 118, 119), (120, 121, 122),
                           (123, 124, 125), (125, 126, 127))[m]
                defer(a, f"o{m}", oproj_mm, 0, m, 0)
                defer(b, f"o{m}", oproj_mm, 0, m, 2)
                defer(f, f"o{m}", oproj_fin, 0, m)

            # ---- flat attention pipeline: 8 jobs x 16 key blocks ----
            jobs = [(qc, pr) for pr in range(PAIRS) for qc in range(2)]
            job_state = {}  # j -> (pA, pB, e_tiles)

            def attnv(j, kb):
                pA, pB, e_tiles = job_state[j]
                e = e_tiles[kb]
                nc.tensor.matmul(pA[0:65, :], vpt[kb][:, 0:65],
                                 e[:, 0:512],
                                 start=(kb == 0), stop=(kb == NKB - 1))
                nc.tensor.matmul(pB[0:65, :], vpt[kb][:, 65:130],
                                 e[:, 512:1024],
                                 start=(kb == 0), stop=(kb == NKB - 1))

            # Normalize pipeline.  pacc is effectively single-buffered per
            # tag, so stage 0 (emitted inline right after the job's last
            # attnv) copies the live [65, 512] accumulator halves to SBUF
            # immediately -- the next job's attnv would otherwise stall on
            # the PSUM buffers.  The rest of the chain (den row to
            # partition 0 by DMA, reciprocal there -- the custom DVE op
            # misbehaves off partition 0 on HW -- GPSIMD partition
            # broadcast, normalize muls) is spread over the following
            # job's slots so no in-order engine stream blocks on its
            # cross-engine latency.
            norm_state = {}

            def norm_s0(j):
                pA, pB, _ = job_state[j]
                cpA = cpool.tile([65, 512], fp32, tag="cpA")
                cpB = cpool.tile([65, 512], fp32, tag="cpB")
                nc.vector.tensor_copy(cpA[:], pA[0:65, :])
                nc.vector.tensor_copy(cpB[:], pB[0:65, :])
                norm_state[j] = [cpA, cpB]

            def norm_s1(j):
                cpA, cpB = norm_state[j]
                d0 = npool.tile([1, 1024], fp32, tag="den0")
                nc.sync.dma_start(d0[0:1, 0:512], cpA[64:65, :])
                nc.sync.dma_start(d0[0:1, 512:1024], cpB[64:65, :])
                norm_state[j] = [cpA, cpB, d0]

            def norm_s2(j):
                cpA, cpB, d0 = norm_state[j]
                r0 = npool.tile([1, 1024], fp32, tag="rden0")
                nc.vector.reciprocal_approx_fast(out=r0[:], in_=d0[:])
                rbcA = bcpool.tile([64, 512], fp32, tag="rbcA")
                rbcB = bcpool.tile([64, 512], fp32, tag="rbcB")
                nc.gpsimd.partition_broadcast(rbcA[:], r0[0:1, 0:512],
                                              channels=64)
                nc.gpsimd.partition_broadcast(rbcB[:], r0[0:1, 512:1024],
                                              channels=64)
                norm_state[j] = [cpA, cpB, rbcA, rbcB]

            def norm_s3(j, last):
                qc, pr = jobs[j]
                cpA, cpB, rbcA, rbcB = norm_state[j]
                ot = opool.tile([128, 512], bf16, tag="ot")
                nc.vector.tensor_mul(ot[0:64, :], cpA[0:64, :], rbcA[:])
                obt = obpool.tile([64, 512], bf16, tag="obt")
                nc.vector.tensor_mul(obt[:], cpB[0:64, :], rbcB[:])
                if not last:
                    # assemble both halves into one [128, 512] tile via an
                    # SBUF->SBUF DMA partition shift so o-proj contracts
                    # over all 128 dims at once
                    nc.sync.dma_start(ot[64:128, :], obt[:])
                    ot_tiles[(qc, pr)] = ot
                else:
                    ot_tiles[(qc, pr)] = (ot, obt)  # unassembled halves

            # normalize stage schedule: jobs 0-5 spread over the next job's
            # slots (chain latency hidden, no engine blocks); job 6 tighter
            # (o-proj qc0 drains need ot(0,3) by slot ~117); job 7 at tail.
            norm_sched = {}
            for j in range(6):
                norm_sched[16 * j + 18] = [
                    (lambda jj: (lambda: norm_s1(jj)))(j)]
                norm_sched[16 * j + 19] = [
                    (lambda jj: (lambda: norm_s2(jj)))(j)]
                norm_sched[16 * j + 21] = [
                    (lambda jj: (lambda: norm_s3(jj, False)))(j)]
            norm_sched[113] = [lambda: norm_s1(6)]
            norm_sched[114] = [lambda: norm_s2(6)]
            norm_sched[115] = [lambda: norm_s3(6, False)]

            for s in range(128):
                j, kb = divmod(s, 16)
                qc, pr = jobs[j]
                if kb == 0:
                    pA = pacc.tile([128, 512], fp32, tag="pacc")
                    pB = pacc.tile([128, 512], fp32, tag="pacc")
                    job_state[j] = (pA, pB, [None] * NKB)

                sc_ps = pssc.tile([128, 1024], fp32, tag="scores")
                nc.tensor.matmul(
                    sc_ps[:, 0:512],
                    ktt[kb // 4][0:64, (kb % 4) * 128:(kb % 4 + 1) * 128],
                    qtt[pr][qc][0:64, :])
                nc.tensor.matmul(
                    sc_ps[:, 512:1024],
                    ktt[kb // 4][64:128, (kb % 4) * 128:(kb % 4 + 1) * 128],
                    qtt[pr][qc][64:128, :])
                e = epool.tile([128, 1024], bf16, tag="E")
                job_state[j][2][kb] = e
                if kb in DVE_SET:
                    nc.vector.tensor_scalar(e[:].bitcast(i16), sc_ps[:],
                                            EXP_A, EXP_B,
                                            AluOpType.mult, AluOpType.add)
                else:
                    nc.scalar.activation(e[:], sc_ps[:], Exp, scale=SCALE)
                if DEBUG_DUMP and s == 0:
                    dbg_e_tile = consts.tile([128, 1024], fp32, tag="dbge")
                    nc.vector.tensor_copy(dbg_e_tile[:], e[:])

                # consume probs with a 2-slot lag so the PE never waits on
                # exp; jobs 6/7 shorten the lag at the job end so their
                # normalize chains (which gate o-proj) start earlier.
                special = {(6, 13), (6, 14), (6, 15),
                           (7, 13), (7, 14), (7, 15)}
                if s >= 2:
                    j2, kb2 = divmod(s - 2, 16)
                    if (j2, kb2) not in special:
                        if kb2 == 0 and j2 >= 1:
                            pass  # deferred one slot (pacc copies in flight)
                        elif kb2 == 1 and j2 >= 1:
                            attnv(j2, 0)
                            attnv(j2, 1)
                        else:
                            attnv(j2, kb2)
                        if kb2 == 15:
                            norm_s0(j2)  # free the PSUM accumulators
                if kb == 15 and j >= 6:
                    attnv(j, 13)  # 2-lag
                    attnv(j, 14)  # 1-lag
                if s == 112:
                    attnv(6, 15)  # 0-lag
                    norm_s0(6)

                for fn in deferred.get(s, ()):
                    fn()
                for fn in norm_sched.get(s, ()):
                    fn()

            # tail: finish job 7, then o-proj for qc1
            attnv(7, 15)
            # job 7 norm: no staging copies (nothing needs pacc freed at
            # the tail) -- den rows go scalar/vector -> SBUF -> DMA shift,
            # muls read the accumulators straight from PSUM.  Dummy
            # matmuls keep the PE p-state up through the chain latency.
            pA7, pB7, _ = job_state[7]
            dst7 = npool.tile([128, 1024], fp32, tag="dstg")
            nc.scalar.copy(dst7[64:65, 0:512], pA7[64:65, :])
            nc.vector.tensor_copy(dst7[64:65, 512:1024], pB7[64:65, :])
            d07 = npool.tile([1, 1024], fp32, tag="den0")
            nc.sync.dma_start(d07[0:1, :], dst7[64:65, :])
            r07 = npool.tile([1, 1024], fp32, tag="rden0")
            nc.vector.reciprocal_approx_fast(out=r07[:], in_=d07[:])
            rbcA7 = bcpool.tile([64, 512], fp32, tag="rbcA")
            rbcB7 = bcpool.tile([64, 512], fp32, tag="rbcB")
            nc.gpsimd.partition_broadcast(rbcA7[:], r07[0:1, 0:512],
                                          channels=64)
            nc.gpsimd.partition_broadcast(rbcB7[:], r07[0:1, 512:1024],
                                          channels=64)
            pe_warm(14)
            otA = opool.tile([128, 512], bf16, tag="ot")
            nc.vector.tensor_mul(otA[0:64, :], pA7[0:64, :], rbcA7[:])
            otB = obpool.tile([64, 512], bf16, tag="obt")
            nc.vector.tensor_mul(otB[:], pB7[0:64, :], rbcB7[:])

            tb = {}
            for m in range(4):
                tb[m] = box = {}
                if m >= 2:
                    box["ps"] = pacc.tile([128, 512], fp32,
                                          name=f"ypt{m}", tag="pacc")
                oproj_mm(1, m, 0, box)  # pr 0,1
            for m in range(4):
                ps = tb[m]["ps"]
                nc.tensor.matmul(ps[:],
                                 ot_tiles[(1, 2)][:, m * 128:(m + 1) * 128],
                                 wo_sb[:, 2, :], start=False, stop=False)
                # pr3 from the unassembled halves as two 64-row matmuls
                # (wo3b is the partition-shifted copy of wo pair-3 rows
                # 64:128) -- skips the ot-assembly DMA on the critical tail
                nc.tensor.matmul(ps[:],
                                 otA[0:64, m * 128:(m + 1) * 128],
                                 wo_sb[0:64, 3, :], start=False, stop=False)
                nc.tensor.matmul(ps[:],
                                 otB[0:64, m * 128:(m + 1) * 128],
                                 wo3b[:], start=False, stop=True)
                oproj_fin(1, m, tb[m])

            if DEBUG_DUMP:
                dbg_vpt = nc.dram_tensor("dbg_vpt", [128, 130], fp32,
                                         kind="ExternalOutput").ap()
                dbg_kt = nc.dram_tensor("dbg_kt", [128, 512], fp32,
                                        kind="ExternalOutput").ap()
                dbg_qt = nc.dram_tensor("dbg_qt", [128, 512], fp32,
                                        kind="ExternalOutput").ap()
                dbg_ot = nc.dram_tensor("dbg_ot", [128, 512], fp32,
                                        kind="ExternalOutput").ap()
                dbg_e = nc.dram_tensor("dbg_e", [128, 1024], fp32,
                                       kind="ExternalOutput").ap()
                tmp = consts.tile([128, 1024], fp32, tag="dbgtmp")
                nc.vector.tensor_copy(tmp[:, 0:130], vpt[0][:])
                nc.sync.dma_start(dbg_vpt, tmp[:, 0:130])
                nc.vector.tensor_copy(tmp[:, 0:512], ktt[0][:])
                nc.sync.dma_start(dbg_kt, tmp[:, 0:512])
                nc.vector.tensor_copy(tmp[:, 0:512], qtt[0][0][:])
                nc.sync.dma_start(dbg_qt, tmp[:, 0:512])
                nc.vector.tensor_copy(tmp[:, 0:512], ot_tiles[(0, 0)][:])
                nc.sync.dma_start(dbg_ot, tmp[:, 0:512])
                nc.vector.tensor_copy(tmp[:], dbg_e_tile[:])
                nc.sync.dma_start(dbg_e, tmp[:])

    nc.finalize()
    return nc


def _get_nc():
    if "nc" not in _built:
        _built["nc"] = _build_nc()
    return _built["nc"]


def _in_maps(x, Wq, bq, Wk, bk, Wv, bv, Wo, bo):
    import ml_dtypes

    b16 = ml_dtypes.bfloat16
    x = np.ascontiguousarray(np.asarray(x, np.float32))
    Wq = np.asarray(Wq, np.float32)
    bq = np.asarray(bq, np.float32)
    Wk = np.asarray(Wk, np.float32)
    bk = np.asarray(bk, np.float32)
    Wv = np.asarray(Wv, np.float32)
    bv = np.asarray(bv, np.float32)
    Wo = np.asarray(Wo, np.float32)
    bo = np.asarray(bo, np.float32)

    def chunked(a):  # [D, n] row-major -> [128, 4*n] with row d = (c, p)
        n = a.shape[1]
        return np.ascontiguousarray(
            a.reshape(4, 128, n).transpose(1, 0, 2).reshape(128, 4 * n))

    wq_p = chunked(
        Wq.reshape(D, H, DH)[:, PERM, :].reshape(D, D)).astype(b16)
    wo_p = chunked(
        Wo.reshape(H, DH, D)[PERM].reshape(D, D)).astype(b16)
    wk_p = chunked(Wk).astype(b16)
    wv_p = chunked(Wv).astype(b16)
    bq_p = np.ascontiguousarray(
        bq.reshape(H, DH)[PERM].reshape(PAIRS, 128).T)
    bqk_p = np.ascontiguousarray(
        np.concatenate([bq_p, bk.reshape(128, 1)], axis=1))
    # fold the v bias through the o-projection: attn@(v+bv) = attn@v + bv
    bv_cat = np.concatenate([bv[(h // 4) * DH:(h // 4 + 1) * DH]
                             for h in range(H)])
    bo_eff = bo + bv_cat @ Wo
    bo_bc = np.ascontiguousarray(np.tile(bo_eff[None, :], (128, 1)))

    in_maps = []
    for c in range(NCORES):
        b, sh = divmod(c, 2)
        xroll = np.roll(x[b], -sh * SQ, axis=0)
        # xT [D, S] -> [128, sc, c, 512] chunk-major contiguous
        xprep = np.ascontiguousarray(
            xroll.T.reshape(4, 128, 4, 512).transpose(1, 2, 0, 3)
            .reshape(128, NSC * 4 * 512)).astype(b16)
        in_maps.append({
            "xp": xprep,
            "wq": wq_p, "wk": wk_p, "wv": wv_p, "wo": wo_p,
            "bqk": bqk_p, "bobc": bo_bc,
        })
    return in_maps


def kernel(x, Wq, bq, Wk, bk, Wv, bv, Wo, bo):
    from concourse.bass_utils import run_bass_kernel_spmd

    in_maps = _in_maps(x, Wq, bq, Wk, bk, Wv, bv, Wo, bo)
    nc = _get_nc()
    res = run_bass_kernel_spmd(nc, in_maps, list(range(NCORES)))
    out = np.empty((B, S, D), np.float32)
    for c in range(NCORES):
        b, sh = divmod(c, 2)
        out[b, sh * SQ:(sh + 1) * SQ, :] = res.results[c]["y"]
    return out
